# revision 21
# baseline (speedup 1.0000x reference)
"""Trainium2 Bass kernel for AttentionFusion (B=4, T=4, H=W=32, C=128).

Sharding: 8 cores = batch (4) x query-half (2). Each core computes full
attention for 2048 query rows of one batch element against all 4096 keys
of that element. No cross-core communication.

v6 design (v4 + prologue overhaul):
  * Wo fused into the V projection host-side: WVo = (Wo Wv xt)^T per tile;
    PV' uses P^T subblocks as the STATIONARY operand and streams
    [ones | WVo] (129 cols): out[q, 0] = rowsum, out[q, 1:129] = y_unnorm.
    No rowsum matmuls, no Wo matmul, per-partition [128,1] reciprocals.
  * Each q-subblock j accumulates in its OWN psum bank (y0..y3) - PSUM
    accumulation groups never interleave within a bank.
  * 5 consolidated input DMAs (per-DMA ring overhead ~1.7us dominated the
    old 12-chunk prologue): packed f32 aux (w3|b2|bo_row), full xs, full
    xt, bf16 wvo, full xtb. Group semaphores a/b/c, waits only on
    full-group counts (DMA completions are unordered).
  * Prologue: Q0-3 + K0-7 proj, then ST chunks 0-5 start as soon as xt
    lands; the WVo projection and the PV' backlog catch up afterwards
    (P^T is buffered for a full block).
  * Flat 64-chunk pipeline (16 x 2-tile exp chunks x 4 blocks) otherwise.
  * Output kept in [q, co] layout; the host transposes.
  * PSUM: ST 2x[128,1024] (4 banks), y0..y3 [128,129] (4 banks).
"""
import sys

sys.path.insert(0, "/opt/trn_rl_repo")

import numpy as np
import ml_dtypes

import concourse.bass as bass
import concourse.mybir as mybir
from concourse.bass_utils import run_bass_kernel_spmd

f32 = mybir.dt.float32
f32r = mybir.dt.float32r
bf16 = mybir.dt.bfloat16

B, T, C, H, W = 4, 4, 128, 32, 32
N = T * H * W            # 4096 keys per batch element
NLOC = N // 2            # 2048 query rows per core
NB = NLOC // 512         # 4 query blocks of 512
MT = N // 128            # 32 key tiles
NCH = NB * 16            # 64 global 2-tile chunks
SCALE = float(C) ** -0.5
N_CORES = 8
PRE = 8                  # ST chunks emitted before PV' catch-up begins
AUXW = 2 * C             # packed aux weights: wqT|wkT


def _build():
    nc = bass.Bass("TRN2")

    xs = nc.declare_dram_parameter("xs", [C, NLOC], f32r, isOutput=False)
    xt = nc.declare_dram_parameter("xt", [C, N], f32r, isOutput=False)
    xtb = nc.declare_dram_parameter("xtb", [C, N], bf16, isOutput=False)
    auxf = nc.declare_dram_parameter("auxf", [C, AUXW], f32r, isOutput=False)
    auxb = nc.declare_dram_parameter("auxb", [C, C + 2], f32, isOutput=False)
    wvo = nc.declare_dram_parameter("wvo", [C, C], bf16, isOutput=False)     # (Wo@Wv)^T
    out = nc.declare_dram_parameter("out", [C, NLOC], f32, isOutput=True)

    # ---- precomputed semaphore schedules (must mirror emission order) ----
    pe = 0
    q_mm, k_mm, wvo_mm = {}, {}, {}
    for j in range(8):
        pe += 1; k_mm[j] = pe
    for j in range(4):
        pe += 1; q_mm[j] = pe
    st_cnt, pv_stop = {}, {}

    def _st_cnt(g):
        nonlocal pe
        for i in range(2):
            pe += 1; st_cnt[(g, i)] = pe

    def _pv_cnt(g):
        nonlocal pe
        pnb, pci = g // 16, g % 16
        for i in range(2):
            for j in range(4):
                pe += 1
                if 2 * pci + i == MT - 1:
                    pv_stop[(pnb, j)] = pe

    for g in range(PRE):
        _st_cnt(g)
        if g >= 2:
            for t in range((g - 2) * 6, min(MT, (g - 2) * 6 + 6)):
                pe += 1; wvo_mm[t] = pe
    pv_ptr = 0
    for g in range(PRE, NCH):
        _st_cnt(g)
        emitted = 0
        while pv_ptr <= g - 1 and emitted < 2:
            _pv_cnt(pv_ptr); pv_ptr += 1; emitted += 1
    while pv_ptr < NCH:
        _pv_cnt(pv_ptr); pv_ptr += 1

    def act_cnt(g):
        return 1 + g

    dve = 0
    dve += MT                       # 32 ones-column memsets
    qcopy, kcopy, wvocopy = {}, {}, {}
    for j in range(8):
        dve += 1; kcopy[j] = dve
    for j in range(4):
        dve += 1; qcopy[j] = dve
    for t in range(MT):
        dve += 1; wvocopy[t] = dve
    y_free = {}
    for nb in range(NB):
        dve += 4                    # recips
        for j in range(4):
            dve += 1; y_free[(nb, j)] = dve  # scalar_tensor_tensor

    from contextlib import ExitStack
    ctx = ExitStack()
    with ctx:
        def sb(name, shape, dt):
            return ctx.enter_context(nc.sbuf_tensor(name, shape, dt))

        def ps(name, shape, dt):
            return ctx.enter_context(nc.psum_tensor(name, shape, dt))

        s_xs = sb("s_xs", [C, NLOC], f32r)
        s_xt = sb("s_xt", [C, N], f32r)
        s_xtb = sb("s_xtb", [C, N], bf16)
        s_auxf = sb("s_auxf", [C, AUXW], f32r)
        s_auxb = sb("s_auxb", [C, C + 2], f32)
        s_wvo = sb("s_wvo", [C, C], bf16)
        s_K = sb("s_K", [C, N], f32r)
        s_Q = sb("s_Q", [C, NLOC], f32r)
        s_WVO = sb("s_WVO", [C, MT * 129], bf16)     # per-tile [ones | WVo]
        s_PT0 = sb("s_PT0", [C, MT * 512], bf16)     # P^T, block ping
        s_PT1 = sb("s_PT1", [C, MT * 512], bf16)     # P^T, block pong
        s_Y = sb("s_Y", [C, NLOC], f32)              # [q, (nb, j, co)]
        s_rc = sb("s_rc", [C, 4], f32)

        st_A = ps("st_A", [C, 1024], f32)            # 2 banks
        st_B = ps("st_B", [C, 1024], f32)            # 2 banks
        ys = [ps(f"y{j}", [C, 129], f32) for j in range(4)]  # 1 bank each

        dma_sem = ctx.enter_context(nc.semaphore("dma_sem"))
        dma_a = ctx.enter_context(nc.semaphore("dma_a"))
        dma_b = ctx.enter_context(nc.semaphore("dma_b"))
        dma_c = ctx.enter_context(nc.semaphore("dma_c"))
        pe_sem = ctx.enter_context(nc.semaphore("pe_sem"))
        act_sem = ctx.enter_context(nc.semaphore("act_sem"))
        dve_sem = ctx.enter_context(nc.semaphore("dve_sem"))
        block = ctx.enter_context(nc.Block())

        w3q = s_auxf[:, 0:C]
        w3k = s_auxf[:, C:2 * C]
        bq_col = s_auxb[:, 0:1]
        bk_col = s_auxb[:, 1:2]
        bo_tile = s_auxb[:, 2:2 + C]

        # prologue st slots: Q0->A0 Q1->A1 Q2->B0 Q3->B1, K0->A0 K1->A1
        # K2->B0 K3->B1 K4->A0 K5->A1 K6->B0 K7->B1
        def st_slot(sl):
            tns = st_A if sl < 2 else st_B
            sub = sl % 2
            return tns[:, sub * 512:(sub + 1) * 512]

        def s_pt(nb):
            return s_PT0 if nb % 2 == 0 else s_PT1

        QDEP = [qcopy[0], qcopy[1], qcopy[2], qcopy[3]]

        @block.tensor
        def _(tensor):
            hi = {}

            def w(sem, name, cnt):
                if cnt > hi.get(name, 0):
                    hi[name] = cnt
                    tensor.wait_ge(sem, cnt)

            for j in range(8):
                w(dma_a, "dma_a", 16 * 2)            # auxf + auxb
                w(dma_b, "dma_b", 16 * 1)            # xt
                if j >= 4:
                    w(dve_sem, "dve", kcopy[j - 4])
                nc.tensor.matmul(st_slot(j % 4), w3k,
                                 s_xt[:, j * 512:(j + 1) * 512],
                                 start=True, stop=True).then_inc(pe_sem, 1)
            for j in range(4):
                w(dma_c, "dma_c", 16 * 1)            # xs
                w(dve_sem, "dve", kcopy[4 + j])
                nc.tensor.matmul(st_slot(j), w3q,
                                 s_xs[:, j * 512:(j + 1) * 512],
                                 start=True, stop=True).then_inc(pe_sem, 1)

            SLOT_FREE = {(0, 0): qcopy[0], (0, 1): qcopy[1],
                         (1, 0): qcopy[2], (1, 1): qcopy[3]}

            def emit_st(g):
                nb = g // 16
                for i in range(2):
                    t = 2 * (g % 16) + i
                    if g >= 2:
                        w(act_sem, "act", act_cnt(g - 2))
                    else:
                        w(dve_sem, "dve", SLOT_FREE[(g, i)])
                    w(dve_sem, "dve", kcopy[t // 4])
                    w(dve_sem, "dve", QDEP[nb])
                    tns = st_A if g % 2 == 0 else st_B
                    nc.tensor.matmul(tns[:, i * 512:(i + 1) * 512],
                                     s_K[:, t * 128:(t + 1) * 128],
                                     s_Q[:, nb * 512:(nb + 1) * 512],
                                     start=True, stop=True).then_inc(pe_sem, 1)

            def emit_pv(g):
                pnb, pci = g // 16, g % 16
                w(act_sem, "act", act_cnt(g))
                if pci == 0 and pnb == 0:
                    w(dve_sem, "dve", wvocopy[MT - 1])
                for i in range(2):
                    t = 2 * pci + i
                    for j in range(4):
                        if pci == 0 and i == 0 and pnb >= 1:
                            w(dve_sem, "dve", y_free[(pnb - 1, j)])
                        nc.tensor.matmul(
                            ys[j][:],
                            s_pt(pnb)[:, t * 512 + j * 128:t * 512 + (j + 1) * 128],
                            s_WVO[:, t * 129:(t + 1) * 129],
                            start=(t == 0), stop=(t == MT - 1),
                            skip_group_check=True).then_inc(pe_sem, 1)

            # WVo proj tiles interleave with ST chunks 0..5
            def emit_wvo(t):
                w(dma_sem, "dma_sem", 16 * 2)        # wvo + xtb
                if t >= 4:
                    w(dve_sem, "dve", wvocopy[t - 4])
                nc.tensor.matmul(ys[t % 4][:, 0:128],
                                 s_xtb[:, t * 128:(t + 1) * 128], s_wvo[:],
                                 start=True, stop=True).then_inc(pe_sem, 1)

            for g in range(PRE):
                emit_st(g)
                if g >= 2:
                    for t in range((g - 2) * 6, min(MT, (g - 2) * 6 + 6)):
                        emit_wvo(t)
            pv_ptr = 0
            for g in range(PRE, NCH):
                emit_st(g)
                emitted = 0
                while pv_ptr <= g - 1 and emitted < 2:
                    emit_pv(pv_ptr); pv_ptr += 1; emitted += 1
            while pv_ptr < NCH:
                emit_pv(pv_ptr); pv_ptr += 1

        @block.scalar
        def _(scalar):
            hi = {}

            def w(sem, name, cnt):
                if cnt > hi.get(name, 0):
                    hi[name] = cnt
                    scalar.wait_ge(sem, cnt)

            for g in range(NCH):
                nb, ci = g // 16, g % 16
                w(pe_sem, "pe", st_cnt[(g, 1)])
                tns = st_A if g % 2 == 0 else st_B
                nc.scalar.activation(s_pt(nb)[:, ci * 1024:(ci + 1) * 1024],
                                     tns[:],
                                     mybir.ActivationFunctionType.Exp,
                                     scale=SCALE).then_inc(act_sem, 1)

        @block.vector
        def _(vector):
            hi = {}

            def w(sem, name, cnt):
                if cnt > hi.get(name, 0):
                    hi[name] = cnt
                    vector.wait_ge(sem, cnt)

            # ones columns of s_WVO
            for t in range(MT):
                vector.memset(s_WVO[:, t * 129:t * 129 + 1], 1.0).then_inc(dve_sem, 1)
            for j in range(8):
                w(pe_sem, "pe", k_mm[j])
                vector.tensor_scalar_add(s_K[:, j * 512:(j + 1) * 512],
                                         st_slot(j % 4), bk_col).then_inc(dve_sem, 1)
            for j in range(4):
                w(pe_sem, "pe", q_mm[j])
                vector.tensor_scalar_add(s_Q[:, j * 512:(j + 1) * 512],
                                         st_slot(j), bq_col).then_inc(dve_sem, 1)
            for t in range(MT):
                w(pe_sem, "pe", wvo_mm[t])
                vector.tensor_copy(s_WVO[:, t * 129 + 1:(t + 1) * 129],
                                   ys[t % 4][:, 0:128]).then_inc(dve_sem, 1)
            # per-block epilogue: y = y_unnorm * (1/rowsum) + bo_eff
            for nb in range(NB):
                for j in range(4):
                    w(pe_sem, "pe", pv_stop[(nb, j)])
                    vector.reciprocal(s_rc[:, j:j + 1],
                                      ys[j][:, 0:1]).then_inc(dve_sem, 1)
                vector.drain()
                for j in range(4):
                    vector.scalar_tensor_tensor(
                        s_Y[:, nb * 512 + j * 128:nb * 512 + (j + 1) * 128],
                        ys[j][:, 1:129], s_rc[:, j:j + 1], bo_tile,
                        mybir.AluOpType.mult,
                        mybir.AluOpType.add).then_inc(dve_sem, 1)

        @block.gpsimd
        def _(gpsimd):
            gpsimd.dma_start(s_auxf[:], auxf[:]).then_inc(dma_a, 16)
            gpsimd.dma_start(s_auxb[:], auxb[:]).then_inc(dma_a, 16)
            gpsimd.dma_start(s_xt[:], xt[:]).then_inc(dma_b, 16)
            gpsimd.dma_start(s_xs[:], xs[:]).then_inc(dma_c, 16)
            gpsimd.dma_start(s_wvo[:], wvo[:]).then_inc(dma_sem, 16)
            gpsimd.dma_start(s_xtb[:], xtb[:]).then_inc(dma_sem, 16)
            for nb in range(NB):
                gpsimd.wait_ge(dve_sem, y_free[(nb, 3)])
                gpsimd.dma_start(out[:, nb * 512:(nb + 1) * 512],
                                 s_Y[:, nb * 512:(nb + 1) * 512]).then_inc(dma_sem, 16)

    return nc


def _make_in_maps(spatial_features, temporal_features, Wq, bq, Wk, bk, Wv, bv, Wo, bo):
    f = np.float32
    bf = ml_dtypes.bfloat16
    bo_eff = (Wo @ bv + bo).astype(f)
    bo_row = np.broadcast_to(bo_eff[None, :], (C, C))
    auxf = np.ascontiguousarray(np.concatenate([Wq.T, Wk.T], axis=1)).astype(f)
    auxb = np.ascontiguousarray(np.concatenate(
        [bq[:, None], bk[:, None], bo_row], axis=1)).astype(f)
    wvo = np.ascontiguousarray((Wo @ Wv).T).astype(bf)

    in_maps = []
    for core in range(N_CORES):
        b, half = core // 2, core % 2
        xs_ = np.ascontiguousarray(
            spatial_features[b, 2 * half:2 * half + 2]      # [2, C, H, W]
            .transpose(1, 0, 2, 3).reshape(C, NLOC)).astype(f)
        xt_ = np.ascontiguousarray(temporal_features[b].reshape(C, N)).astype(f)
        in_maps.append({
            "xs": xs_,
            "xt": xt_,
            "xtb": xt_.astype(bf),
            "auxf": auxf,
            "auxb": auxb,
            "wvo": wvo,
        })
    return in_maps


def _assemble(res):
    out = np.empty((B, C, T, H, W), np.float32)
    for core in range(N_CORES):
        b, half = core // 2, core % 2
        y = np.asarray(res.results[core]["out"])            # [128, (nb, j, co)]
        y = y.reshape(128, NB, 4, C).transpose(1, 2, 0, 3).reshape(NLOC, C)
        out[b, :, 2 * half:2 * half + 2] = y.T.reshape(C, 2, H, W)
    return out


_CACHED = {}


def _run(in_maps, trace=False):
    if "nc" not in _CACHED:
        _CACHED["nc"] = _build()
    return run_bass_kernel_spmd(_CACHED["nc"], in_maps, list(range(N_CORES)), trace=trace)


def kernel(spatial_features, temporal_features, Wq, bq, Wk, bk, Wv, bv, Wo, bo):
    args = [np.asarray(a) for a in (spatial_features, temporal_features,
                                    Wq, bq, Wk, bk, Wv, bv, Wo, bo)]
    in_maps = _make_in_maps(*args)
    res = _run(in_maps)
    return _assemble(res)


# revision 22
# speedup vs baseline: 1.0161x; 1.0161x over previous
"""Trainium2 Bass kernel for AttentionFusion (B=4, T=4, H=W=32, C=128).

Sharding: 8 cores = batch (4) x query-half (2). Each core computes full
attention for 2048 query rows of one batch element against all 4096 keys
of that element. No cross-core communication.

v6 design (v4 + prologue overhaul):
  * Wo fused into the V projection host-side: WVo = (Wo Wv xt)^T per tile;
    PV' uses P^T subblocks as the STATIONARY operand and streams
    [ones | WVo] (129 cols): out[q, 0] = rowsum, out[q, 1:129] = y_unnorm.
    No rowsum matmuls, no Wo matmul, per-partition [128,1] reciprocals.
  * Each q-subblock j accumulates in its OWN psum bank (y0..y3) - PSUM
    accumulation groups never interleave within a bank.
  * 5 consolidated input DMAs (per-DMA ring overhead ~1.7us dominated the
    old 12-chunk prologue): packed f32 aux (w3|b2|bo_row), full xs, full
    xt, bf16 wvo, full xtb. Group semaphores a/b/c, waits only on
    full-group counts (DMA completions are unordered).
  * Prologue: Q0-3 + K0-7 proj, then ST chunks 0-5 start as soon as xt
    lands; the WVo projection and the PV' backlog catch up afterwards
    (P^T is buffered for a full block).
  * Flat 64-chunk pipeline (16 x 2-tile exp chunks x 4 blocks) otherwise.
  * Output kept in [q, co] layout; the host transposes.
  * PSUM: ST 2x[128,1024] (4 banks), y0..y3 [128,129] (4 banks).
"""
import sys

sys.path.insert(0, "/opt/trn_rl_repo")

import numpy as np
import ml_dtypes

import concourse.bass as bass
import concourse.mybir as mybir
from concourse.bass_utils import run_bass_kernel_spmd

f32 = mybir.dt.float32
f32r = mybir.dt.float32r
bf16 = mybir.dt.bfloat16

B, T, C, H, W = 4, 4, 128, 32, 32
N = T * H * W            # 4096 keys per batch element
NLOC = N // 2            # 2048 query rows per core
NB = NLOC // 512         # 4 query blocks of 512
MT = N // 128            # 32 key tiles
NCH = NB * 16            # 64 global 2-tile chunks
SCALE = float(C) ** -0.5
N_CORES = 8
PRE = 6                  # ST chunks emitted before the WVo projection
AUXW = 2 * C             # packed aux weights: wqT|wkT


def _build():
    nc = bass.Bass("TRN2")

    xs = nc.declare_dram_parameter("xs", [C, NLOC], f32r, isOutput=False)
    xt = nc.declare_dram_parameter("xt", [C, N], f32r, isOutput=False)
    xtb = nc.declare_dram_parameter("xtb", [C, N], bf16, isOutput=False)
    auxf = nc.declare_dram_parameter("auxf", [C, AUXW], f32r, isOutput=False)
    auxb = nc.declare_dram_parameter("auxb", [C, C + 2], f32, isOutput=False)
    wvo = nc.declare_dram_parameter("wvo", [C, C], bf16, isOutput=False)     # (Wo@Wv)^T
    out = nc.declare_dram_parameter("out", [C, NLOC], f32, isOutput=True)

    # ---- precomputed semaphore schedules (must mirror emission order) ----
    pe = 0
    q_mm, k_mm, wvo_mm = {}, {}, {}
    for j in range(4):
        pe += 1; q_mm[j] = pe
    for j in range(8):
        pe += 1; k_mm[j] = pe
    st_cnt, pv_stop = {}, {}

    def _st_cnt(g):
        nonlocal pe
        for i in range(2):
            pe += 1; st_cnt[(g, i)] = pe

    def _pv_cnt(g):
        nonlocal pe
        pnb, pci = g // 16, g % 16
        for i in range(2):
            for j in range(4):
                pe += 1
                if 2 * pci + i == MT - 1:
                    pv_stop[(pnb, j)] = pe

    for g in range(PRE):
        _st_cnt(g)
        for t in range(g * 6, min(MT, g * 6 + 6)):
            pe += 1; wvo_mm[t] = pe
    pv_ptr = 0
    for g in range(PRE, NCH):
        _st_cnt(g)
        emitted = 0
        while pv_ptr <= g - 1 and emitted < 2:
            _pv_cnt(pv_ptr); pv_ptr += 1; emitted += 1
    while pv_ptr < NCH:
        _pv_cnt(pv_ptr); pv_ptr += 1

    def act_cnt(g):
        return 1 + g

    dve = 0
    dve += MT                       # 32 ones-column memsets
    qcopy, kcopy, wvocopy = {}, {}, {}
    for j in range(4):
        dve += 1; qcopy[j] = dve
    for j in range(8):
        dve += 1; kcopy[j] = dve
    for t in range(MT):
        dve += 1; wvocopy[t] = dve
    y_free = {}
    for nb in range(NB):
        dve += 4                    # recips
        dve += 4; y_free[nb] = dve  # scalar_tensor_tensor x4

    from contextlib import ExitStack
    ctx = ExitStack()
    with ctx:
        def sb(name, shape, dt):
            return ctx.enter_context(nc.sbuf_tensor(name, shape, dt))

        def ps(name, shape, dt):
            return ctx.enter_context(nc.psum_tensor(name, shape, dt))

        s_xs = sb("s_xs", [C, NLOC], f32r)
        s_xt = sb("s_xt", [C, N], f32r)
        s_xtb = sb("s_xtb", [C, N], bf16)
        s_auxf = sb("s_auxf", [C, AUXW], f32r)
        s_auxb = sb("s_auxb", [C, C + 2], f32)
        s_wvo = sb("s_wvo", [C, C], bf16)
        s_K = sb("s_K", [C, N], f32r)
        s_Q = sb("s_Q", [C, NLOC], f32r)
        s_WVO = sb("s_WVO", [C, MT * 129], bf16)     # per-tile [ones | WVo]
        s_PT0 = sb("s_PT0", [C, MT * 512], bf16)     # P^T, block ping
        s_PT1 = sb("s_PT1", [C, MT * 512], bf16)     # P^T, block pong
        s_Y = sb("s_Y", [C, NLOC], f32)              # [q, (nb, j, co)]
        s_rc = sb("s_rc", [C, 4], f32)

        st_A = ps("st_A", [C, 1024], f32)            # 2 banks
        st_B = ps("st_B", [C, 1024], f32)            # 2 banks
        ys = [ps(f"y{j}", [C, 129], f32) for j in range(4)]  # 1 bank each

        dma_sem = ctx.enter_context(nc.semaphore("dma_sem"))
        dma_a = ctx.enter_context(nc.semaphore("dma_a"))
        dma_b = ctx.enter_context(nc.semaphore("dma_b"))
        dma_c = ctx.enter_context(nc.semaphore("dma_c"))
        pe_sem = ctx.enter_context(nc.semaphore("pe_sem"))
        act_sem = ctx.enter_context(nc.semaphore("act_sem"))
        dve_sem = ctx.enter_context(nc.semaphore("dve_sem"))
        block = ctx.enter_context(nc.Block())

        w3q = s_auxf[:, 0:C]
        w3k = s_auxf[:, C:2 * C]
        bq_col = s_auxb[:, 0:1]
        bk_col = s_auxb[:, 1:2]
        bo_tile = s_auxb[:, 2:2 + C]

        # prologue st slots: Q0->A0 Q1->A1 Q2->B0 Q3->B1, K0->A0 K1->A1
        # K2->B0 K3->B1 K4->A0 K5->A1 K6->B0 K7->B1
        def st_slot(sl):
            tns = st_A if sl < 2 else st_B
            sub = sl % 2
            return tns[:, sub * 512:(sub + 1) * 512]

        def s_pt(nb):
            return s_PT0 if nb % 2 == 0 else s_PT1

        QDEP = [qcopy[0], qcopy[1], qcopy[2], qcopy[3]]

        @block.tensor
        def _(tensor):
            hi = {}

            def w(sem, name, cnt):
                if cnt > hi.get(name, 0):
                    hi[name] = cnt
                    tensor.wait_ge(sem, cnt)

            for j in range(4):
                w(dma_a, "dma_a", 16 * 3)            # auxf + auxb + xs
                nc.tensor.matmul(st_slot(j), w3q,
                                 s_xs[:, j * 512:(j + 1) * 512],
                                 start=True, stop=True).then_inc(pe_sem, 1)
            for j in range(8):
                w(dma_b, "dma_b", 16 * 1)            # xt
                prev = {0: qcopy[0], 1: qcopy[1], 2: qcopy[2], 3: qcopy[3],
                        4: kcopy[0], 5: kcopy[1], 6: kcopy[2], 7: kcopy[3]}[j]
                w(dve_sem, "dve", prev)
                nc.tensor.matmul(st_slot(j % 4), w3k,
                                 s_xt[:, j * 512:(j + 1) * 512],
                                 start=True, stop=True).then_inc(pe_sem, 1)

            SLOT_FREE = {(0, 0): kcopy[4], (0, 1): kcopy[5],
                         (1, 0): kcopy[6], (1, 1): kcopy[7]}

            def emit_st(g):
                nb = g // 16
                for i in range(2):
                    t = 2 * (g % 16) + i
                    if g >= 2:
                        w(act_sem, "act", act_cnt(g - 2))
                    else:
                        w(dve_sem, "dve", SLOT_FREE[(g, i)])
                    w(dve_sem, "dve", kcopy[t // 4])
                    w(dve_sem, "dve", QDEP[nb])
                    tns = st_A if g % 2 == 0 else st_B
                    nc.tensor.matmul(tns[:, i * 512:(i + 1) * 512],
                                     s_K[:, t * 128:(t + 1) * 128],
                                     s_Q[:, nb * 512:(nb + 1) * 512],
                                     start=True, stop=True).then_inc(pe_sem, 1)

            def emit_pv(g):
                pnb, pci = g // 16, g % 16
                w(act_sem, "act", act_cnt(g))
                if pci == 0:
                    w(dve_sem, "dve",
                      y_free[pnb - 1] if pnb >= 1 else wvocopy[MT - 1])
                for i in range(2):
                    t = 2 * pci + i
                    for j in range(4):
                        nc.tensor.matmul(
                            ys[j][:],
                            s_pt(pnb)[:, t * 512 + j * 128:t * 512 + (j + 1) * 128],
                            s_WVO[:, t * 129:(t + 1) * 129],
                            start=(t == 0), stop=(t == MT - 1),
                            skip_group_check=True).then_inc(pe_sem, 1)

            # WVo proj tiles interleave with ST chunks 0..5
            def emit_wvo(t):
                w(dma_c, "dma_c", 16 * 2)            # wvo + xtb
                if t >= 4:
                    w(dve_sem, "dve", wvocopy[t - 4])
                nc.tensor.matmul(ys[t % 4][:, 0:128],
                                 s_xtb[:, t * 128:(t + 1) * 128], s_wvo[:],
                                 start=True, stop=True).then_inc(pe_sem, 1)

            for g in range(PRE):
                emit_st(g)
                for t in range(g * 6, min(MT, g * 6 + 6)):
                    emit_wvo(t)
            pv_ptr = 0
            for g in range(PRE, NCH):
                emit_st(g)
                emitted = 0
                while pv_ptr <= g - 1 and emitted < 2:
                    emit_pv(pv_ptr); pv_ptr += 1; emitted += 1
            while pv_ptr < NCH:
                emit_pv(pv_ptr); pv_ptr += 1

        @block.scalar
        def _(scalar):
            hi = {}

            def w(sem, name, cnt):
                if cnt > hi.get(name, 0):
                    hi[name] = cnt
                    scalar.wait_ge(sem, cnt)

            for g in range(NCH):
                nb, ci = g // 16, g % 16
                w(pe_sem, "pe", st_cnt[(g, 1)])
                tns = st_A if g % 2 == 0 else st_B
                nc.scalar.activation(s_pt(nb)[:, ci * 1024:(ci + 1) * 1024],
                                     tns[:],
                                     mybir.ActivationFunctionType.Exp,
                                     scale=SCALE).then_inc(act_sem, 1)

        @block.vector
        def _(vector):
            hi = {}

            def w(sem, name, cnt):
                if cnt > hi.get(name, 0):
                    hi[name] = cnt
                    vector.wait_ge(sem, cnt)

            # ones columns of s_WVO
            for t in range(MT):
                vector.memset(s_WVO[:, t * 129:t * 129 + 1], 1.0).then_inc(dve_sem, 1)
            for j in range(4):
                w(pe_sem, "pe", q_mm[j])
                vector.tensor_scalar_add(s_Q[:, j * 512:(j + 1) * 512],
                                         st_slot(j), bq_col).then_inc(dve_sem, 1)
            for j in range(8):
                w(pe_sem, "pe", k_mm[j])
                vector.tensor_scalar_add(s_K[:, j * 512:(j + 1) * 512],
                                         st_slot(j % 4), bk_col).then_inc(dve_sem, 1)
            for t in range(MT):
                w(pe_sem, "pe", wvo_mm[t])
                vector.tensor_copy(s_WVO[:, t * 129 + 1:(t + 1) * 129],
                                   ys[t % 4][:, 0:128]).then_inc(dve_sem, 1)
            # per-block epilogue: y = y_unnorm * (1/rowsum) + bo_eff
            for nb in range(NB):
                for j in range(4):
                    w(pe_sem, "pe", pv_stop[(nb, j)])
                    vector.reciprocal(s_rc[:, j:j + 1],
                                      ys[j][:, 0:1]).then_inc(dve_sem, 1)
                vector.drain()
                for j in range(4):
                    vector.scalar_tensor_tensor(
                        s_Y[:, nb * 512 + j * 128:nb * 512 + (j + 1) * 128],
                        ys[j][:, 1:129], s_rc[:, j:j + 1], bo_tile,
                        mybir.AluOpType.mult,
                        mybir.AluOpType.add).then_inc(dve_sem, 1)

        @block.gpsimd
        def _(gpsimd):
            gpsimd.dma_start(s_auxf[:], auxf[:]).then_inc(dma_a, 16)
            gpsimd.dma_start(s_auxb[:], auxb[:]).then_inc(dma_a, 16)
            gpsimd.dma_start(s_xs[:], xs[:]).then_inc(dma_a, 16)
            gpsimd.dma_start(s_xt[:], xt[:]).then_inc(dma_b, 16)
            gpsimd.dma_start(s_wvo[:], wvo[:]).then_inc(dma_c, 16)
            gpsimd.dma_start(s_xtb[:], xtb[:]).then_inc(dma_c, 16)
            for nb in range(NB):
                gpsimd.wait_ge(dve_sem, y_free[nb])
                gpsimd.dma_start(out[:, nb * 512:(nb + 1) * 512],
                                 s_Y[:, nb * 512:(nb + 1) * 512]).then_inc(dma_sem, 16)

    return nc


def _make_in_maps(spatial_features, temporal_features, Wq, bq, Wk, bk, Wv, bv, Wo, bo):
    f = np.float32
    bf = ml_dtypes.bfloat16
    bo_eff = (Wo @ bv + bo).astype(f)
    bo_row = np.broadcast_to(bo_eff[None, :], (C, C))
    auxf = np.ascontiguousarray(np.concatenate([Wq.T, Wk.T], axis=1)).astype(f)
    auxb = np.ascontiguousarray(np.concatenate(
        [bq[:, None], bk[:, None], bo_row], axis=1)).astype(f)
    wvo = np.ascontiguousarray((Wo @ Wv).T).astype(bf)

    in_maps = []
    for core in range(N_CORES):
        b, half = core // 2, core % 2
        xs_ = np.ascontiguousarray(
            spatial_features[b, 2 * half:2 * half + 2]      # [2, C, H, W]
            .transpose(1, 0, 2, 3).reshape(C, NLOC)).astype(f)
        xt_ = np.ascontiguousarray(temporal_features[b].reshape(C, N)).astype(f)
        in_maps.append({
            "xs": xs_,
            "xt": xt_,
            "xtb": xt_.astype(bf),
            "auxf": auxf,
            "auxb": auxb,
            "wvo": wvo,
        })
    return in_maps


def _assemble(res):
    out = np.empty((B, C, T, H, W), np.float32)
    for core in range(N_CORES):
        b, half = core // 2, core % 2
        y = np.asarray(res.results[core]["out"])            # [128, (nb, j, co)]
        y = y.reshape(128, NB, 4, C).transpose(1, 2, 0, 3).reshape(NLOC, C)
        out[b, :, 2 * half:2 * half + 2] = y.T.reshape(C, 2, H, W)
    return out


_CACHED = {}


def _run(in_maps, trace=False):
    if "nc" not in _CACHED:
        _CACHED["nc"] = _build()
    return run_bass_kernel_spmd(_CACHED["nc"], in_maps, list(range(N_CORES)), trace=trace)


def kernel(spatial_features, temporal_features, Wq, bq, Wk, bk, Wv, bv, Wo, bo):
    args = [np.asarray(a) for a in (spatial_features, temporal_features,
                                    Wq, bq, Wk, bk, Wv, bv, Wo, bo)]
    in_maps = _make_in_maps(*args)
    res = _run(in_maps)
    return _assemble(res)
